# revision 31
# baseline (speedup 1.0000x reference)
"""Causal multi-head attention on 8 Trainium2 NeuronCores.

Problem: x[2,4096,512], W_q/W_k/W_v/W_proj[512,512], b_proj[512]
  q,k,v = x @ W.T split into 8 heads of 64; causal softmax(q k^T / 8) v;
  out = attn @ W_proj.T + b_proj.

Sharding: 16 (batch, head) pairs over 8 cores -> each core gets one batch
and a pair of adjacent heads (128 of the 512 hidden dims).  The output
projection is computed per-core against the matching 128-row slice of
W_proj^T, giving a partial [4096, 512] output per core; the host sums the
4 partials per batch and adds the bias.

On-core design (bf16 datapath, fp32 PSUM accumulation):
  xT   [512, 4096] bf16 (host-transposed input slice)
  qT/kT [128, 4096] bf16: rows 0-63 head0, 64-127 head1
  scores: per head a [128 k, 1024 (2 k-blocks x 512 q)] PSUM tile; the two
  heads' score matmuls are K=64 row-tiles at partition offsets 0/64 and run
  concurrently on the PE sub-arrays
  exp on ScalarE (scale=1/8 folded in) -> ex bf16 in SBUF; causal masking of
  diagonal blocks via DVE mul with precomputed 0/1 masks
  attnT accumulated as [65, 512] PSUM per head: rows 0-63 v^T ex, row 64
  the softmax denominators (ones column appended to v)
  normalize: DVE reciprocal of the denominator row -> K=1 rank-1 matmul
  broadcasts 1/den to [64, 512] -> copy+mul write normalized bf16 attnT
  out = attnT^T @ wpT per 128-row block -> fp32 partial DMA'd out

Emission order is software-pipelined for the strict-FIFO engine queues:
the attn@v matmuls for k-block group i are emitted one group late (so PE
never waits on ScalarE), and the projection work of chunk qc+1 plus the
normalize/output work of chunk qc-1 are spread round-robin into chunk qc's
k-block groups (so ScalarE never drains at chunk boundaries).
"""

import numpy as np

B, S, D, H = 2, 4096, 512, 8
DH = 64
QCHUNK = 512
SCALE = 1.0 / np.sqrt(DH)

_CACHE = {}


def _build(s=S, repeats=1):
    from contextlib import ExitStack

    import concourse.mybir as mybir
    import concourse.tile as tile
    from concourse import bacc

    f32 = mybir.dt.float32
    f32r = mybir.dt.float32r
    bf16 = mybir.dt.bfloat16

    ndc = D // 128
    nkb_all = s // 128

    nc = bacc.Bacc("TRN2")
    xT_d = nc.dram_tensor("xT", [D, s], bf16, kind="ExternalInput")
    wpack_d = nc.dram_tensor("wpack", [128, 4160], bf16, kind="ExternalInput")
    ones65_d = nc.dram_tensor("ones_f32", [97, 64], f32r, kind="ExternalInput")
    out_d = nc.dram_tensor("out_p", [s, D], f32, kind="ExternalOutput")

    with ExitStack() as ctx:
        tc = ctx.enter_context(tile.TileContext(nc))
        consts = ctx.enter_context(tc.tile_pool(name="consts", bufs=1))
        big = ctx.enter_context(tc.tile_pool(name="big", bufs=1))
        expool = ctx.enter_context(tc.tile_pool(name="expool", bufs=3))
        recpool = ctx.enter_context(tc.tile_pool(name="recpool", bufs=2))
        outpool = ctx.enter_context(tc.tile_pool(name="outpool", bufs=2))
        mmps = ctx.enter_context(tc.tile_pool(name="mmps", bufs=2, space="PSUM"))
        scps = ctx.enter_context(tc.tile_pool(name="scps", bufs=1, space="PSUM"))
        accps = ctx.enter_context(tc.tile_pool(name="accps", bufs=1, space="PSUM"))

        # ---- persistent SBUF ----
        xfull = big.tile([128, ndc * s], bf16, name="xfull", tag="xfull")
        xT = [xfull[:, c * s:(c + 1) * s] for c in range(ndc)]
        qT = big.tile([128, s], bf16, name="qT", tag="qT")
        kT = big.tile([128, s], bf16, name="kT", tag="kT")
        v65 = [big.tile([128, 65 * nkb_all], bf16, name=f"v65_{h}", tag=f"v65_{h}")
               for h in range(2)]
        attnT = big.tile([128, s], bf16, name="attnT", tag="attnT")
        wpack = consts.tile([128, 4160], bf16, name="wpack", tag="wpack")
        xq0 = [wpack[:, 1024 + c * QCHUNK:1024 + (c + 1) * QCHUNK]
               for c in range(ndc)]
        wq = wpack[:, 0:512]
        wk = wpack[:, 512:1024]
        wv = wpack[:, 3072:3584]
        wp = wpack[:, 3584:4096]
        warmsb = consts.tile([64, 512], bf16, name="warmsb", tag="warmsb")
        masks4 = [consts.tile([128, 512], bf16, name="mask0", tag="mask0")]
        ones65 = consts.tile([97, 64], f32r, name="ones65", tag="ones65")

        env = dict(locals())
        for _rep in range(repeats):
            _emit_body(nc, tc, env, first=(_rep == 0))

    nc.compile()
    return nc


def _emit_body(nc, tc, env, first=True):
    """One full pass of the kernel body (DMAs + all chunks)."""
    import concourse.mybir as mybir

    f32 = mybir.dt.float32
    f32r = mybir.dt.float32r
    bf16 = mybir.dt.bfloat16
    EXP = mybir.ActivationFunctionType.Exp
    GE = mybir.AluOpType.is_ge

    s, ndc = env["s"], env["ndc"]
    nqc = s // QCHUNK
    nkb_all = s // 128
    (xT_d, wpack_d, ones65_d, out_d) = (
        env["xT_d"], env["wpack_d"], env["ones65_d"], env["out_d"])
    (xfull, xT, xq0, qT, kT, v65, attnT, wpack, wq, wk, wv, wp, masks4,
     ones65, warmsb) = (
        env["xfull"], env["xT"], env["xq0"], env["qT"], env["kT"], env["v65"],
        env["attnT"], env["wpack"], env["wq"], env["wk"], env["wv"],
        env["wp"], env["masks4"], env["ones65"], env["warmsb"])
    (expool, recpool, outpool, mmps, scps, accps) = (
        env["expool"], env["recpool"], env["outpool"], env["mmps"],
        env["scps"], env["accps"])

    # ---- input DMAs + constants ----
    # order: packed weight image first, then the first q-chunk's slice of x,
    # then the bulk of x — so chunk 0's projections start ~2us in
    # PE warmup: dummy matmuls on scratch keep the HAM clock gate from
    # throttling the first real projections (and cost nothing -- they run
    # during the input DMA)
    nc.vector.memset(warmsb, 0.0)
    for _w in range(7):
        wps = mmps.tile([128, QCHUNK], f32, name=f"warm{_w}", tag="mm")
        nc.tensor.matmul(wps, lhsT=warmsb[:, 0:128], rhs=warmsb,
                         start=True, stop=True)
    nc.sync.dma_start(out=wpack[:, 0:3072], in_=wpack_d[:, 0:3072])
    nc.sync.dma_start(out=wpack[:, 3072:4160], in_=wpack_d[:, 3072:4160])
    xin = xT_d.rearrange("(c p) j -> p c j", p=128)
    xsb = xfull.rearrange("p (c j) -> p c j", j=s)
    nc.sync.dma_start(out=xsb[:, :, QCHUNK:2 * QCHUNK],
                      in_=xin[:, :, QCHUNK:2 * QCHUNK])
    if first:  # input-independent constants: build once, reuse across bodies
        nc.sync.dma_start(out=ones65, in_=ones65_d.ap())
        for h in range(2):
            ones_ap = v65[h].rearrange("p (k c) -> p k c", c=65)[:, :, 64]
            nc.sync.dma_start(out=ones_ap, in_=wpack_d[:, 4096:4096 + nkb_all])
    for lo, hi in ((2 * QCHUNK, min(4 * QCHUNK, s)), (4 * QCHUNK, s)):
        if hi > lo:
            nc.sync.dma_start(out=xsb[:, :, lo:hi], in_=xin[:, :, lo:hi])
    if first:
        # mask[p, f] = 1.0 where f >= p else 0.0 (shared by all diagonal
        # blocks after narrowing -- each is a lower triangle in its slice)
        nc.gpsimd.memset(masks4[0], 1.0)
        nc.gpsimd.affine_select(
            out=masks4[0], in_=masks4[0], compare_op=GE, fill=0.0,
            base=0, channel_multiplier=-1, pattern=[[1, 512]])

    acc_tiles = {}

    def proj_units(qc, split_v=False):
        """Projection work for chunk qc: [q, k, v0..v3] emission units."""
        qlo = qc * QCHUNK
        qs = slice(qlo, qlo + QCHUNK)

        def xsrc(c):  # chunk 0's x slice rides in the wpack image
            return xq0[c] if qc == 0 else xT[c][:, qs]

        def qk_unit(w_sb, dst):
            def emit():
                ps = mmps.tile([128, QCHUNK], f32, name=f"proj_{qc}", tag="mm")
                for c in range(ndc):
                    nc.tensor.matmul(ps,
                                     lhsT=w_sb[:, c * 128:(c + 1) * 128],
                                     rhs=xsrc(c),
                                     start=(c == 0), stop=(c == ndc - 1))
                if qc == 0:  # ScalarE is idle during the lead-in
                    nc.scalar.copy(dst[:, qs], ps)
                else:
                    nc.vector.tensor_copy(dst[:, qs], ps)
            return emit

        def v_unit(j):
            def emit():
                kb = qc * 4 + j
                vp = mmps.tile([128, 128], f32, name=f"vp_{kb}", tag="mm")
                for c in range(ndc):
                    nc.tensor.matmul(vp,
                                     lhsT=xsrc(c)[:, j * 128:(j + 1) * 128],
                                     rhs=wv[:, c * 128:(c + 1) * 128],
                                     start=(c == 0), stop=(c == ndc - 1))
                for h in range(2):
                    nc.vector.tensor_copy(v65[h][:, kb * 65:kb * 65 + 64],
                                          vp[:, h * 64:(h + 1) * 64])
            return emit

        if split_v:
            return [qk_unit(wq, qT), qk_unit(wk, kT)], [v_unit(j) for j in range(4)]
        return [qk_unit(wq, qT), qk_unit(wk, kT)] + [v_unit(j) for j in range(4)]

    def norm_out_units(qc):
        """Normalize + output projection for chunk qc: 6 emission units.
        The first two (normalize h0/h1) free the acc PSUM banks and must be
        emitted before chunk qc+1's first attn-accumulate matmul."""
        qlo = qc * QCHUNK
        qs = slice(qlo, qlo + QCHUNK)
        acc = acc_tiles[qc]
        ot = [None]
        final = qc == nqc - 1

        def norm_unit():
            def emit():
                recs, bcs = [], []
                for h in range(2):
                    # h0's reciprocal row lives at partition 64, h1's at 96:
                    # the two K=1 broadcast matmuls then sit on disjoint PE
                    # row groups and stream concurrently
                    row = 64 + 32 * h
                    rec = recpool.tile([97, QCHUNK], f32r, name=f"rec{h}_{qc}",
                                       tag="rec")
                    with nc.allow_low_precision(
                            reason="softmax 1/den broadcast"):
                        nc.vector.reciprocal(rec[row:row + 1, :],
                                             acc[h][64:65, :])
                    recs.append((rec, row))
                for h in range(2):
                    rec, row = recs[h]
                    bc = mmps.tile([128, QCHUNK], f32, name=f"bc{h}_{qc}",
                                   tag="mm")
                    nc.tensor.matmul(bc[0:64, :], lhsT=ones65[row:row + 1, :],
                                     rhs=rec[row:row + 1, :],
                                     tile_position=(row, 0),
                                     start=True, stop=True)
                    bcs.append(bc)
                for h in range(2):
                    hsl = slice(h * 64, (h + 1) * 64)
                    if final:
                        nc.scalar.copy(attnT[hsl, qs], acc[h][0:64, :])
                    else:
                        nc.vector.tensor_copy(attnT[hsl, qs], acc[h][0:64, :])
                    nc.vector.tensor_mul(attnT[hsl, qs], attnT[hsl, qs],
                                         bcs[h][0:64, :])
            return emit

        def out_unit(j):
            def emit():
                qb = qc * 4 + j
                pp = mmps.tile([128, D], f32, name=f"pp_{qb}", tag="mm")
                nc.tensor.matmul(pp,
                                 lhsT=attnT[:, qb * 128:(qb + 1) * 128],
                                 rhs=wp, start=True, stop=True)
                if ot[0] is None:
                    ot[0] = outpool.tile([128, 4 * D], f32, name=f"ot_{qc}",
                                         tag="ot")
                if final:
                    nc.scalar.copy(ot[0][:, j * D:(j + 1) * D], pp)
                else:
                    nc.vector.tensor_copy(ot[0][:, j * D:(j + 1) * D], pp)
                if final:  # tail: per-block DMAs pipeline with the copies
                    nc.sync.dma_start(out=out_d[qb * 128:(qb + 1) * 128, :],
                                      in_=ot[0][:, j * D:(j + 1) * D])
                elif j == 3:
                    od = out_d[qc * QCHUNK:(qc + 1) * QCHUNK, :]
                    nc.sync.dma_start(
                        out=od.rearrange("(j p) d -> p j d", p=128),
                        in_=ot[0].rearrange("p (j d) -> p j d", d=D))
            return emit

        return [norm_unit()] + [out_unit(j) for j in range(4)]

    # prologue: chunk 0's projections
    for u in proj_units(0):
        u()

    vprev = []
    for qc in range(nqc):
        qlo = qc * QCHUNK
        qs = slice(qlo, qlo + QCHUNK)
        nkb = (qc + 1) * 4
        ngrp = nkb // 2

        # units interleaved into this chunk's k-block groups
        pinned, rest = [], []
        if qc > 0:
            no = norm_out_units(qc - 1)
            pinned = no[:1]           # normalize unit: must precede acc start
            rest = no[1:]
        vrest = []
        if qc + 1 < nqc:
            qk, vu = proj_units(qc + 1, split_v=True)
            rest = rest + qk
            # v65 blocks of chunk qc+1 are only read by its last two k-block
            # groups, so its v-projections can ride in its own early groups
            vrest = vu

        acc = [accps.tile([65, QCHUNK], f32, name=f"acc{h}_{qc}", tag=f"acc{h}")
               for h in range(2)]
        acc_tiles[qc] = acc
        ex_tiles = {}

        def blocks_of(i):
            """(kb, offset-in-tile, width, q-offset) for group i's 2 k-blocks.
            Diagonal blocks are narrowed to their valid (unmasked) q-columns:
            block r of the chunk's diagonal only reaches q >= qlo + 128*r."""
            out = []
            for j in range(2):
                kb = i * 2 + j
                r = kb - qc * 4
                w = QCHUNK - 128 * r if r >= 0 else QCHUNK
                # offsets stay 512-aligned: each score matmul owns a full
                # PSUM bank (two start=True matmuls must not share a bank --
                # the second one's bank clear corrupts the first on HW)
                out.append((kb, j * QCHUNK, w, QCHUNK - w))
            return out

        def emit_acc(i):
            for h in range(2):
                ex = ex_tiles.pop((i, h))
                for kb, off, w, qoff in blocks_of(i):
                    nc.tensor.matmul(
                        acc[h][:, qoff:QCHUNK],
                        lhsT=v65[h][:, kb * 65:(kb + 1) * 65],
                        rhs=ex[:, off:off + w],
                        start=(kb == 0), stop=(kb == nkb - 1))

        for i in range(ngrp):
            blocks = blocks_of(i)
            tot = max(off + w for _, off, w, _ in blocks)
            scs = [scps.tile([128, 1024], f32, name=f"sc{h}_{qc}_{i}",
                             tag=f"sc{h}") for h in range(2)]
            # j-outer, h-inner: adjacent matmuls sit on disjoint PE row
            # groups (partitions 0-63 vs 64-127) and stream concurrently
            for kb, off, w, qoff in blocks:
                for h in range(2):
                    hsl = slice(h * 64, (h + 1) * 64)
                    nc.tensor.matmul(
                        scs[h][:, off:off + w],
                        lhsT=kT[hsl, kb * 128:(kb + 1) * 128],
                        rhs=qT[hsl, qlo + qoff:qlo + QCHUNK],
                        start=True, stop=True)
            for h in range(2):
                sc = scs[h]
                ex = expool.tile([128, 1024], bf16, name=f"ex{h}_{qc}_{i}",
                                 tag=f"ex{h}")
                (kb0, off0, w0, _), (kb1, off1, w1, _) = blocks
                if off0 + w0 == off1:  # contiguous: one activation
                    nc.scalar.activation(ex[:, 0:tot], sc[:, 0:tot], EXP,
                                         scale=float(SCALE))
                else:
                    nc.scalar.activation(ex[:, off0:off0 + w0],
                                         sc[:, off0:off0 + w0], EXP,
                                         scale=float(SCALE))
                    nc.scalar.activation(ex[:, off1:off1 + w1],
                                         sc[:, off1:off1 + w1], EXP,
                                         scale=float(SCALE))
                ex_tiles[(i, h)] = ex
            for h in range(2):
                for kb, off, w, qoff in blocks:
                    if kb * 128 >= qlo:  # diagonal block: zero where k > q
                        ex = ex_tiles[(i, h)]
                        sl = ex[:, off:off + w]
                        nc.vector.tensor_mul(sl, sl, masks4[0][:, 0:w])
            if i == 0:
                for u in pinned:
                    u()
            nrest = len(rest)
            for u in rest[i * nrest // ngrp:(i + 1) * nrest // ngrp]:
                u()
            nv = len(vprev)
            vg = max(1, ngrp - 2)
            if i < vg:
                for u in vprev[i * nv // vg:(i + 1) * nv // vg]:
                    u()
            if i > 0:
                emit_acc(i - 1)
        emit_acc(ngrp - 1)
        vprev = vrest

    for u in norm_out_units(nqc - 1):
        u()


def _in_maps(x, W_q, W_k, W_v, W_proj):
    import ml_dtypes
    bf16 = ml_dtypes.bfloat16

    def sbuf_image(wT):  # [512, 128] -> [128, 512] SBUF image (4 K-chunks)
        return wT.reshape(4, 128, 128).transpose(1, 0, 2).reshape(128, 512)

    maps = []
    for c in range(8):
        b, hp = c // 4, c % 4
        cols = slice(hp * 128, (hp + 1) * 128)
        xTb = x[b].T
        x0img = xTb[:, 0:512].reshape(4, 128, 512).transpose(1, 0, 2).reshape(128, 2048)
        wpack = np.concatenate([
            sbuf_image(W_q.T[:, cols]),
            sbuf_image(W_k.T[:, cols]),
            x0img,
            sbuf_image(W_v.T[:, cols]),
            W_proj[:, cols].T,
            np.ones((128, 64), dtype=np.float32),
        ], axis=1).astype(bf16)
        maps.append({
            "xT": np.ascontiguousarray(x[b].T).astype(bf16),
            "wpack": np.ascontiguousarray(wpack),
            "ones_f32": np.ones((97, 64), dtype=np.float32),
        })
    return maps


def kernel(x, W_q, W_k, W_v, W_proj, b_proj, _trace=False):
    from concourse.bass_utils import run_bass_kernel_spmd

    x = np.asarray(x, dtype=np.float32)
    W_q = np.asarray(W_q, dtype=np.float32)
    W_k = np.asarray(W_k, dtype=np.float32)
    W_v = np.asarray(W_v, dtype=np.float32)
    W_proj = np.asarray(W_proj, dtype=np.float32)
    b_proj = np.asarray(b_proj, dtype=np.float32)

    if "nc" not in _CACHE:
        _CACHE["nc"] = _build()
    nc = _CACHE["nc"]

    res = run_bass_kernel_spmd(nc, _in_maps(x, W_q, W_k, W_v, W_proj),
                               core_ids=list(range(8)), trace=_trace)
    out = np.empty((B, S, D), dtype=np.float32)
    for b in range(B):
        acc = res.results[4 * b]["out_p"].astype(np.float32)
        for j in range(1, 4):
            acc = acc + res.results[4 * b + j]["out_p"]
        out[b] = acc + b_proj
    if _trace:
        _CACHE["last_trace"] = res
    return out
